# revision 1
# baseline (speedup 1.0000x reference)
"""Grouped-experts SwiGLU MLP (DeepseekV3 style) for Trainium2, 8 NeuronCores.

Sharding: expert-parallel. Core e owns expert e's weights and its static
4096-token split. No collectives needed — token routing is the host-side
slice, outputs concatenate back in token order.

Per-core kernel (all matmuls in bf16 with fp32 PSUM accumulation):
  gT[h, t] = wg[d, h].T @ xT[d, t]      (accumulate over 16 d-chunks of 128)
  uT[h, t] = wu[d, h].T @ xT[d, t]
  hT[h, t] = silu(gT) * uT              (ACT silu + DVE mul, stored bf16)
  out[t, d] = hT[h, t].T @ wd[h, d]     (accumulate over 11 h-chunks of 128)

x is fed pre-transposed ([dim, tokens]) per core so the contraction dim sits
on SBUF partitions for both operands; weights are DMA-cast fp32->bf16 on
load and stay resident in SBUF for the whole kernel.
"""

import numpy as np

NUM_EXPERTS = 8
DIM = 2048
HIDDEN = 1408
T_E = 4096  # tokens per expert (static equal splits)

P = 128
TN = 512              # token group width (matmul moving dim)
NG = T_E // TN        # 8 token groups
DC = DIM // P         # 16 contraction chunks for the up/gate matmuls
HC = HIDDEN // P      # 11 contraction chunks for the down matmul
NDO = DIM // TN       # 4 output-dim blocks of 512

_nc_cache = []


def _build_program():
    import concourse.mybir as mybir
    import concourse.tile as tile
    from concourse import bacc

    fp32 = mybir.dt.float32
    bf16 = mybir.dt.bfloat16
    AF = mybir.ActivationFunctionType

    nc = bacc.Bacc("TRN2", target_bir_lowering=False, debug=False)

    xT = nc.dram_tensor("xt", [DIM, T_E], fp32, kind="ExternalInput")
    wg = nc.dram_tensor("wg", [DIM, HIDDEN], fp32, kind="ExternalInput")
    wu = nc.dram_tensor("wu", [DIM, HIDDEN], fp32, kind="ExternalInput")
    wd = nc.dram_tensor("wd", [HIDDEN, DIM], fp32, kind="ExternalInput")
    out = nc.dram_tensor("out", [T_E, DIM], fp32, kind="ExternalOutput")

    with tile.TileContext(nc) as tc:
        with (
            tc.tile_pool(name="wpool", bufs=1) as wpool,
            # xt double-buffered: group g+1's 16 cast-DMAs (~28us incl SWDGE
            # emission) hide under mm1/2(g) (~97us) instead of only mm3(g)
            # (~24us at real HW matmul rates) — removes a per-group PE stall.
            tc.tile_pool(name="xpool", bufs=2) as xpool,
            tc.tile_pool(name="hpool", bufs=1) as hpool,
            tc.tile_pool(name="spool", bufs=1) as spool,
            tc.tile_pool(name="opool", bufs=1) as opool,
            tc.tile_pool(name="psum", bufs=2, space="PSUM") as psum_pool,
        ):
            # Resident bf16 weights: [128, chunk, free] with the contraction
            # chunk index as the middle dim. DMA-cast fp32->bf16 (SWDGE).
            wg_sb = wpool.tile([P, DC, HIDDEN], bf16, tag="wg")
            wu_sb = wpool.tile([P, DC, HIDDEN], bf16, tag="wu")
            wd_sb = wpool.tile([P, HC, DIM], bf16, tag="wd")
            # Emission order matters for the SWDGE queue: the first matmul
            # needs wg + xt(group 0); wu is read ~3us later, wd not until
            # the first down-projection (~75us in). Load in that order.
            xt0_sb = xpool.tile([P, DC, TN], bf16, tag="xt")
            for c in range(DC):
                nc.gpsimd.dma_start(out=wg_sb[:, c, :], in_=wg[c * P:(c + 1) * P, :])
            for c in range(DC):
                nc.gpsimd.dma_start(out=xt0_sb[:, c, :], in_=xT[c * P:(c + 1) * P, 0:TN])
            for c in range(DC):
                nc.gpsimd.dma_start(out=wu_sb[:, c, :], in_=wu[c * P:(c + 1) * P, :])
            for c in range(HC):
                nc.gpsimd.dma_start(out=wd_sb[:, c, :], in_=wd[c * P:(c + 1) * P, :])

            for g in range(NG):
                # xT group [128, 16, 512] bf16, DMA-cast per d-chunk.
                if g == 0:
                    xt_sb = xt0_sb
                else:
                    xt_sb = xpool.tile([P, DC, TN], bf16, tag="xt")
                    for c in range(DC):
                        nc.gpsimd.dma_start(
                            out=xt_sb[:, c, :],
                            in_=xT[c * P:(c + 1) * P, g * TN:(g + 1) * TN],
                        )

                ht_sb = hpool.tile([P, HC, TN], bf16, tag="ht")
                for hh in range(HC):
                    pg = psum_pool.tile([P, TN], fp32, tag="pg")
                    pu = psum_pool.tile([P, TN], fp32, tag="pu")
                    for c in range(DC):
                        nc.tensor.matmul(
                            pg,
                            wg_sb[:, c, hh * P:(hh + 1) * P],
                            xt_sb[:, c, :],
                            start=(c == 0),
                            stop=(c == DC - 1),
                        )
                    for c in range(DC):
                        nc.tensor.matmul(
                            pu,
                            wu_sb[:, c, hh * P:(hh + 1) * P],
                            xt_sb[:, c, :],
                            start=(c == 0),
                            stop=(c == DC - 1),
                        )
                    # silu(g)*u = (g * sigmoid(g)) * u. Each DVE op reads at
                    # most one PSUM operand (HW limit NCC_IBVF027); Silu LUT
                    # isn't in CoreSim so sigmoid+mul keeps this sim-testable.
                    sig = spool.tile([P, TN], fp32, tag="sig")
                    sil = spool.tile([P, TN], fp32, tag="sil")
                    nc.scalar.activation(sig, pg, AF.Sigmoid)
                    nc.vector.tensor_mul(sil, pg, sig)
                    nc.vector.tensor_mul(ht_sb[:, hh, :], sil, pu)

                for tb in range(TN // P):
                    ot = opool.tile([P, DIM], fp32, tag="ot")
                    # hh-outer so one stationary hT load feeds 4 accumulating
                    # matmuls (one per dout block) -> 4x fewer LDWEIGHTS.
                    # po spans 4 PSUM banks; pg/pu take the other 4.
                    po = psum_pool.tile([P, NDO, TN], fp32, tag="po", bufs=1)
                    for hh in range(HC):
                        for do in range(NDO):
                            nc.tensor.matmul(
                                po[:, do, :],
                                ht_sb[:, hh, tb * P:(tb + 1) * P],
                                wd_sb[:, hh, do * TN:(do + 1) * TN],
                                start=(hh == 0),
                                stop=(hh == HC - 1),
                            )
                    for do in range(NDO):
                        nc.vector.tensor_copy(ot[:, do * TN:(do + 1) * TN], po[:, do, :])
                    t0 = g * TN + tb * P
                    nc.sync.dma_start(out=out[t0:t0 + P, :], in_=ot)

    nc.compile()
    return nc


def _get_program():
    if not _nc_cache:
        _nc_cache.append(_build_program())
    return _nc_cache[0]


def kernel(x, num_tokens_per_expert, w_gate, w_up, w_down, **_ignored):
    from concourse.bass_utils import run_bass_kernel_spmd

    x = np.asarray(x, dtype=np.float32)
    w_gate = np.asarray(w_gate, dtype=np.float32)
    w_up = np.asarray(w_up, dtype=np.float32)
    w_down = np.asarray(w_down, dtype=np.float32)

    nc = _get_program()

    xe = x.reshape(NUM_EXPERTS, T_E, DIM)
    in_maps = []
    for e in range(NUM_EXPERTS):
        in_maps.append(
            {
                "xt": np.ascontiguousarray(xe[e].T),
                "wg": np.ascontiguousarray(w_gate[e]),
                "wu": np.ascontiguousarray(w_up[e]),
                "wd": np.ascontiguousarray(w_down[e]),
            }
        )

    res = run_bass_kernel_spmd(nc, in_maps, core_ids=list(range(NUM_EXPERTS)))
    outs = [np.asarray(r["out"], dtype=np.float32) for r in res.results]
    return np.concatenate(outs, axis=0)



# revision 2
# speedup vs baseline: 1.0336x; 1.0336x over previous
"""Grouped-experts SwiGLU MLP (DeepseekV3 style) for Trainium2, 8 NeuronCores.

Sharding: expert-parallel. Core e owns expert e's weights and its static
4096-token split. No collectives needed — token routing is the host-side
slice, outputs concatenate back in token order.

Per-core kernel (all matmuls in bf16 with fp32 PSUM accumulation):
  gT[h, t] = wg[d, h].T @ xT[d, t]      (accumulate over 16 d-chunks of 128)
  uT[h, t] = wu[d, h].T @ xT[d, t]
  hT[h, t] = silu(gT) * uT              (ACT sigmoid + DVE muls, stored bf16)
  out[t, d] = hT[h, t].T @ wd[h, d]     (accumulate over 11 h-chunks of 128)

The PE is the bottleneck (bf16 matmul roofline ~900us/core; fp8 DoubleRow
fails the 2e-2 accuracy budget — measured 3.8-6.6% rel err on this problem).
So everything else is shaped to keep the PE issue stream stall-free:

- All HBM tensors are bf16, cast host-side (identical numerics to the old
  fp32->bf16 DMA cast path, RNE both ways). Halves startup DMA bytes: the
  PE's first accumulation chain needs wg + x(group 0), 7.7 MB instead of
  15.7 MB, and the full weight set streams in 48us instead of 97us.
- Inputs land in per-contraction-chunk tiles (16 separate wg/wu/x tiles)
  DMA'd in consumption order, wg/x interleaved across two issue engines, so
  the first matmul starts ~2us in and group 0 is paced by DMA arrival
  rather than blocked on the whole tensor.
- Output is bf16 (adds ~0.02% rel err) with a double-buffered out tile:
  the last group's PSUM->SBUF->HBM drain no longer serializes against the
  final down-projection matmuls.
"""

import numpy as np

NUM_EXPERTS = 8
DIM = 2048
HIDDEN = 1408
T_E = 4096  # tokens per expert (static equal splits)

P = 128
TN = 512              # token group width (matmul moving dim)
NG = T_E // TN        # 8 token groups
DC = DIM // P         # 16 contraction chunks for the up/gate matmuls
HC = HIDDEN // P      # 11 contraction chunks for the down matmul
NDO = DIM // TN       # 4 output-dim blocks of 512

_nc_cache = []


def _build_program():
    import concourse.mybir as mybir
    import concourse.tile as tile
    from concourse import bacc

    fp32 = mybir.dt.float32
    bf16 = mybir.dt.bfloat16
    AF = mybir.ActivationFunctionType

    nc = bacc.Bacc("TRN2", target_bir_lowering=False, debug=False)

    xt = nc.dram_tensor("xt", [NG, DC, P, TN], bf16, kind="ExternalInput")
    wg = nc.dram_tensor("wg", [DC, P, HIDDEN], bf16, kind="ExternalInput")
    wu = nc.dram_tensor("wu", [DC, P, HIDDEN], bf16, kind="ExternalInput")
    wd = nc.dram_tensor("wd", [HC, P, DIM], bf16, kind="ExternalInput")
    out = nc.dram_tensor("out", [T_E, DIM], bf16, kind="ExternalOutput")

    with tile.TileContext(nc) as tc:
        with (
            tc.tile_pool(name="wpool", bufs=1) as wpool,
            tc.tile_pool(name="xpool", bufs=2) as xpool,
            tc.tile_pool(name="hpool", bufs=1) as hpool,
            tc.tile_pool(name="spool", bufs=1) as spool,
            tc.tile_pool(name="opool", bufs=2) as opool,
            tc.tile_pool(name="psum", bufs=2, space="PSUM") as psum_pool,
        ):
            wg_sb = [
                wpool.tile([P, HIDDEN], bf16, tag=f"wg{c}", name=f"wg{c}")
                for c in range(DC)
            ]
            wu_sb = [
                wpool.tile([P, HIDDEN], bf16, tag=f"wu{c}", name=f"wu{c}")
                for c in range(DC)
            ]
            wd_sb = wpool.tile([P, HC, DIM], bf16, tag="wd")
            x0_sb = [
                xpool.tile([P, TN], bf16, tag=f"x{c}", name=f"x0_{c}")
                for c in range(DC)
            ]
            # Consumption-ordered loads. The first accumulation chain (hh=0)
            # reads wg chunk c then x0 chunk c, so interleave those pairs,
            # split across two issue engines so descriptor emission doesn't
            # pace the queue. wu is consumed ~3.4us after wg chunk parity,
            # wd not until the first down-projection (~75us of PE time in).
            for c in range(DC):
                nc.gpsimd.dma_start(out=wg_sb[c], in_=wg[c])
                nc.sync.dma_start(out=x0_sb[c], in_=xt[0, c])
            for c in range(DC):
                nc.gpsimd.dma_start(out=wu_sb[c], in_=wu[c])
            for h in range(HC):
                nc.sync.dma_start(out=wd_sb[:, h, :], in_=wd[h])

            for g in range(NG):
                if g == 0:
                    x_sb = x0_sb
                else:
                    x_sb = [
                        xpool.tile([P, TN], bf16, tag=f"x{c}", name=f"x{g}_{c}")
                        for c in range(DC)
                    ]
                    for c in range(DC):
                        nc.gpsimd.dma_start(out=x_sb[c], in_=xt[g, c])

                ht_sb = hpool.tile([P, HC, TN], bf16, tag="ht")
                for hh in range(HC):
                    pg = psum_pool.tile([P, TN], fp32, tag="pg")
                    pu = psum_pool.tile([P, TN], fp32, tag="pu")
                    for c in range(DC):
                        nc.tensor.matmul(
                            pg,
                            wg_sb[c][:, hh * P:(hh + 1) * P],
                            x_sb[c],
                            start=(c == 0),
                            stop=(c == DC - 1),
                        )
                    for c in range(DC):
                        nc.tensor.matmul(
                            pu,
                            wu_sb[c][:, hh * P:(hh + 1) * P],
                            x_sb[c],
                            start=(c == 0),
                            stop=(c == DC - 1),
                        )
                    # silu(g)*u = (g * sigmoid(g)) * u. Each DVE op reads at
                    # most one PSUM operand (HW limit NCC_IBVF027); Silu LUT
                    # isn't in CoreSim so sigmoid+mul keeps this sim-testable.
                    sig = spool.tile([P, TN], fp32, tag="sig")
                    sil = spool.tile([P, TN], fp32, tag="sil")
                    nc.scalar.activation(sig, pg, AF.Sigmoid)
                    nc.vector.tensor_mul(sil, pg, sig)
                    nc.vector.tensor_mul(ht_sb[:, hh, :], sil, pu)

                for tb in range(TN // P):
                    ot = opool.tile([P, DIM], bf16, tag="ot")
                    # hh-outer so one stationary hT load feeds 4 accumulating
                    # matmuls (one per dout block) -> 4x fewer LDWEIGHTS.
                    # po spans 4 PSUM banks; pg/pu take the other 4.
                    po = psum_pool.tile([P, NDO, TN], fp32, tag="po", bufs=1)
                    for hh in range(HC):
                        for do in range(NDO):
                            nc.tensor.matmul(
                                po[:, do, :],
                                ht_sb[:, hh, tb * P:(tb + 1) * P],
                                wd_sb[:, hh, do * TN:(do + 1) * TN],
                                start=(hh == 0),
                                stop=(hh == HC - 1),
                            )
                    for do in range(NDO):
                        nc.vector.tensor_copy(ot[:, do * TN:(do + 1) * TN], po[:, do, :])
                    t0 = g * TN + tb * P
                    nc.sync.dma_start(out=out[t0:t0 + P, :], in_=ot)

    nc.compile()
    return nc


def _get_program():
    if not _nc_cache:
        _nc_cache.append(_build_program())
    return _nc_cache[0]


def _in_map_for_core(xe, w_gate, w_up, w_down, e):
    import ml_dtypes

    bf = ml_dtypes.bfloat16
    xtc = np.ascontiguousarray(xe[e].T).astype(bf)           # [DIM, T_E]
    xtc = np.ascontiguousarray(
        xtc.reshape(DC, P, NG, TN).transpose(2, 0, 1, 3)     # [NG, DC, P, TN]
    )
    return {
        "xt": xtc,
        "wg": np.ascontiguousarray(w_gate[e].astype(bf).reshape(DC, P, HIDDEN)),
        "wu": np.ascontiguousarray(w_up[e].astype(bf).reshape(DC, P, HIDDEN)),
        "wd": np.ascontiguousarray(w_down[e].astype(bf).reshape(HC, P, DIM)),
    }


def kernel(x, num_tokens_per_expert, w_gate, w_up, w_down, **_ignored):
    from concourse.bass_utils import run_bass_kernel_spmd

    x = np.asarray(x, dtype=np.float32)
    w_gate = np.asarray(w_gate, dtype=np.float32)
    w_up = np.asarray(w_up, dtype=np.float32)
    w_down = np.asarray(w_down, dtype=np.float32)

    nc = _get_program()

    xe = x.reshape(NUM_EXPERTS, T_E, DIM)
    in_maps = [
        _in_map_for_core(xe, w_gate, w_up, w_down, e) for e in range(NUM_EXPERTS)
    ]

    res = run_bass_kernel_spmd(nc, in_maps, core_ids=list(range(NUM_EXPERTS)))
    outs = [np.asarray(r["out"]).astype(np.float32) for r in res.results]
    return np.concatenate(outs, axis=0)


# revision 3
# speedup vs baseline: 1.0523x; 1.0181x over previous
"""Grouped-experts SwiGLU MLP (DeepseekV3 style) for Trainium2, 8 NeuronCores.

Sharding: expert-parallel. Core e owns expert e's weights and its static
4096-token split. No collectives needed — token routing is the host-side
slice, outputs concatenate back in token order.

Per-core kernel (all matmuls in bf16 with fp32 PSUM accumulation):
  gT[h, t] = wg[d, h].T @ xT[d, t]      (accumulate over 16 d-chunks of 128)
  uT[h, t] = wu[d, h].T @ xT[d, t]
  hT[h, t] = silu(gT) * uT              (ACT sigmoid + DVE muls, stored bf16)
  out[t, d] = hT[h, t].T @ wd[h, d]     (accumulate over 11 h-chunks of 128)

The PE is the bottleneck (bf16 matmul roofline ~900us/core; fp8 DoubleRow
fails the 2e-2 accuracy budget — measured 3.8-6.6% rel err on this problem).
So everything else is shaped to keep the PE issue stream stall-free:

- All HBM tensors are bf16, cast host-side (identical numerics to the old
  fp32->bf16 DMA cast path, RNE both ways). Halves startup DMA bytes: the
  PE's first accumulation chain needs wg + x(group 0), 7.7 MB instead of
  15.7 MB, and the full weight set streams in 48us instead of 97us.
- Inputs land in per-contraction-chunk tiles (16 separate wg/wu/x tiles)
  DMA'd in consumption order, wg/x interleaved across two issue engines, so
  the first matmul starts ~2us in and group 0 is paced by DMA arrival
  rather than blocked on the whole tensor.
- Output is bf16 (adds ~0.02% rel err) with a double-buffered out tile:
  the last group's PSUM->SBUF->HBM drain no longer serializes against the
  final down-projection matmuls.
"""

import numpy as np

NUM_EXPERTS = 8
DIM = 2048
HIDDEN = 1408
T_E = 4096  # tokens per expert (static equal splits)

P = 128
TN = 512              # token group width (matmul moving dim)
NG = T_E // TN        # 8 token groups
DC = DIM // P         # 16 contraction chunks for the up/gate matmuls
HC = HIDDEN // P      # 11 contraction chunks for the down matmul
NDO = DIM // TN       # 4 output-dim blocks of 512

_nc_cache = []


def _build_program():
    import concourse.mybir as mybir
    import concourse.tile as tile
    from concourse import bacc

    fp32 = mybir.dt.float32
    bf16 = mybir.dt.bfloat16
    AF = mybir.ActivationFunctionType

    nc = bacc.Bacc("TRN2", target_bir_lowering=False, debug=False)

    xt = nc.dram_tensor("xt", [NG, DC, P, TN], bf16, kind="ExternalInput")
    wg = nc.dram_tensor("wg", [DC, P, HIDDEN], bf16, kind="ExternalInput")
    wu = nc.dram_tensor("wu", [DC, P, HIDDEN], bf16, kind="ExternalInput")
    wd = nc.dram_tensor("wd", [HC, P, DIM], bf16, kind="ExternalInput")
    out = nc.dram_tensor("out", [T_E, DIM], bf16, kind="ExternalOutput")

    with tile.TileContext(nc) as tc:
        with (
            tc.tile_pool(name="wpool", bufs=1) as wpool,
            tc.tile_pool(name="xpool", bufs=2) as xpool,
            tc.tile_pool(name="hpool", bufs=1) as hpool,
            tc.tile_pool(name="spool", bufs=1) as spool,
            tc.tile_pool(name="opool", bufs=2) as opool,
            tc.tile_pool(name="psum", bufs=2, space="PSUM") as psum_pool,
        ):
            wg_sb = [
                wpool.tile([P, HIDDEN], bf16, tag=f"wg{c}", name=f"wg{c}")
                for c in range(DC)
            ]
            wu_sb = [
                wpool.tile([P, HIDDEN], bf16, tag=f"wu{c}", name=f"wu{c}")
                for c in range(DC)
            ]
            wd_sb = wpool.tile([P, HC, DIM], bf16, tag="wd")
            x0_sb = [
                xpool.tile([P, TN], bf16, tag=f"x{c}", name=f"x0_{c}")
                for c in range(DC)
            ]
            # Consumption-ordered loads. The first accumulation chain (hh=0)
            # reads wg chunk c then x0 chunk c, so interleave those pairs,
            # split across two issue engines so descriptor emission doesn't
            # pace the queue. wu is consumed ~3.4us after wg chunk parity,
            # wd not until the first down-projection (~75us of PE time in).
            for c in range(DC):
                nc.gpsimd.dma_start(out=wg_sb[c], in_=wg[c])
                nc.sync.dma_start(out=x0_sb[c], in_=xt[0, c])
            for c in range(DC):
                nc.gpsimd.dma_start(out=wu_sb[c], in_=wu[c])
            for h in range(HC):
                nc.sync.dma_start(out=wd_sb[:, h, :], in_=wd[h])

            for g in range(NG):
                if g == 0:
                    x_sb = x0_sb
                else:
                    x_sb = [
                        xpool.tile([P, TN], bf16, tag=f"x{c}", name=f"x{g}_{c}")
                        for c in range(DC)
                    ]
                    for c in range(DC):
                        nc.gpsimd.dma_start(out=x_sb[c], in_=xt[g, c])

                ht_sb = hpool.tile([P, HC, TN], bf16, tag="ht")
                for hh in range(HC):
                    pg = psum_pool.tile([P, TN], fp32, tag="pg")
                    pu = psum_pool.tile([P, TN], fp32, tag="pu")
                    for c in range(DC):
                        nc.tensor.matmul(
                            pg,
                            wg_sb[c][:, hh * P:(hh + 1) * P],
                            x_sb[c],
                            start=(c == 0),
                            stop=(c == DC - 1),
                        )
                    for c in range(DC):
                        nc.tensor.matmul(
                            pu,
                            wu_sb[c][:, hh * P:(hh + 1) * P],
                            x_sb[c],
                            start=(c == 0),
                            stop=(c == DC - 1),
                        )
                    # silu(g)*u = (g * sigmoid(g)) * u. Each DVE op reads at
                    # most one PSUM operand (HW limit NCC_IBVF027); Silu LUT
                    # isn't in CoreSim so sigmoid+mul keeps this sim-testable.
                    sig = spool.tile([P, TN], fp32, tag="sig")
                    sil = spool.tile([P, TN], fp32, tag="sil")
                    nc.scalar.activation(sig, pg, AF.Sigmoid)
                    nc.vector.tensor_mul(sil, pg, sig)
                    nc.vector.tensor_mul(ht_sb[:, hh, :], sil, pu)

                # Down-projection in dim-halves: po spans 2 PSUM banks and is
                # double-buffered (4 banks total; pg/pu take the other 4), so
                # the PSUM->bf16 casts and the out store of one half overlap
                # the matmuls of the next instead of stalling the PE at every
                # tb boundary (LDWEIGHTS is emitted 1:1 per matmul by the
                # framework, so the extra half split costs no weight loads).
                HD = DIM // 2
                for tb in range(TN // P):
                    t0 = g * TN + tb * P
                    for half in range(2):
                        ot = opool.tile([P, HD], bf16, tag="ot", name=f"ot{g}_{tb}_{half}")
                        po = psum_pool.tile([P, 2, TN], fp32, tag="po")
                        for hh in range(HC):
                            for do in range(2):
                                nc.tensor.matmul(
                                    po[:, do, :],
                                    ht_sb[:, hh, tb * P:(tb + 1) * P],
                                    wd_sb[:, hh, (half * 2 + do) * TN:(half * 2 + do + 1) * TN],
                                    start=(hh == 0),
                                    stop=(hh == HC - 1),
                                )
                        for do in range(2):
                            nc.vector.tensor_copy(ot[:, do * TN:(do + 1) * TN], po[:, do, :])
                        nc.sync.dma_start(
                            out=out[t0:t0 + P, half * HD:(half + 1) * HD], in_=ot
                        )

    nc.compile()
    return nc


def _get_program():
    if not _nc_cache:
        _nc_cache.append(_build_program())
    return _nc_cache[0]


def _in_map_for_core(xe, w_gate, w_up, w_down, e):
    import ml_dtypes

    bf = ml_dtypes.bfloat16
    xtc = np.ascontiguousarray(xe[e].T).astype(bf)           # [DIM, T_E]
    xtc = np.ascontiguousarray(
        xtc.reshape(DC, P, NG, TN).transpose(2, 0, 1, 3)     # [NG, DC, P, TN]
    )
    return {
        "xt": xtc,
        "wg": np.ascontiguousarray(w_gate[e].astype(bf).reshape(DC, P, HIDDEN)),
        "wu": np.ascontiguousarray(w_up[e].astype(bf).reshape(DC, P, HIDDEN)),
        "wd": np.ascontiguousarray(w_down[e].astype(bf).reshape(HC, P, DIM)),
    }


def kernel(x, num_tokens_per_expert, w_gate, w_up, w_down, **_ignored):
    from concourse.bass_utils import run_bass_kernel_spmd

    x = np.asarray(x, dtype=np.float32)
    w_gate = np.asarray(w_gate, dtype=np.float32)
    w_up = np.asarray(w_up, dtype=np.float32)
    w_down = np.asarray(w_down, dtype=np.float32)

    nc = _get_program()

    xe = x.reshape(NUM_EXPERTS, T_E, DIM)
    in_maps = [
        _in_map_for_core(xe, w_gate, w_up, w_down, e) for e in range(NUM_EXPERTS)
    ]

    res = run_bass_kernel_spmd(nc, in_maps, core_ids=list(range(NUM_EXPERTS)))
    outs = [np.asarray(r["out"]).astype(np.float32) for r in res.results]
    return np.concatenate(outs, axis=0)
